# revision 11
# baseline (speedup 1.0000x reference)
"""Kalman filter estimator (nn_KalmanFilterEstimator) as a Bass/Tile kernel on 8 TRN2 cores.

Reformulation: the scan is linear in the data once the (data-independent) Riccati
gain sequence is known. With x0 = 0:

    x_{t+1} = x_t @ Aeff_t + c_t,
    c_t     = u_t @ (B_W G_t) + d_t @ (E_W G_t) + ym_t @ Lc_t^T,
    G_t     = I - C_W @ Lc_t^T,   Aeff_t = A_W @ G_t,

so x_T = sum_t c_t @ (Aeff_{t+1} ... Aeff_{T-1}).  The gain converges to Lbar in
~46 steps (rho(Abar) ~ 0.73, checked at runtime), so Aeff_t == Abar beyond the
first few steps and the suffix product is Abar^(T-1-t).  Contributions decay as
rho^age: anything older than ~330 steps underflows to exactly 0 in float32 (the
reference output provably cannot depend on it).  We therefore compute

    x_T = sum_{t >= T-WIN} c_t @ Abar^(T-1-t),        WIN = 256
        (truncation error ~ rho^WIN ~ 4e-35  <<  f32 epsilon)

time-sharded over 8 cores (32 steps each).  Per core m, with 16-step blocks:

    partial_m = sum_{kl<2} [ sum_{q<16} Z_{t(kl,q)} @ W_{15-q} ] @ MB_{m,kl}
    W_a      = [B_W G; E_W G; Lbar^T] @ Abar^a            ([128 x 128], stacked)
    MB_{m,kl} = Abar^(16 (1-kl) + 32 (7-m))
    Z_t      = [u_t ; d_t ; ym_t] transposed to [128 feat x 128 batch]

All device work is K=128 matmuls accumulated in PSUM (inner stage fuses the two
blocks into N=256 moving operands); the 8 [NX x B] partials are summed on host.
Weight-only precompute (Riccati, matrix powers) runs on host in float64.
"""

import numpy as np

NX, NY, NU, ND = 128, 64, 32, 32
T, B = 2048, 128
HEAT_C = 0.997 * 4185.5 * (1.0 / 3600.0)
N_CORES = 8
WIN = 128                  # time window that fully determines x_T at f32
TCW = WIN // N_CORES       # 16 timesteps per core
NA = 8                     # inner radix (Abar^a, a in [0,8)) = block length
NBW = TCW // NA            # 2 blocks of 8 steps per core
_cache = {}


def _build_weights(A_W, B_W, E_W, C_W, Q, R, P0, L0):
    """Riccati recursion in float64 -> folded steady-state weights (f32)."""
    A = A_W.astype(np.float64); C = C_W.astype(np.float64)
    Qf = Q.astype(np.float64); Rf = R.astype(np.float64)
    eye = np.eye(NX)
    P = P0.astype(np.float64); L = L0.astype(np.float64)
    prev = None
    for t in range(300):
        P_pred = A @ P @ A.T + Qf
        S = Rf + C.T @ P_pred @ C
        L = P_pred @ C @ np.linalg.inv(S)
        P = eye - L @ (C.T @ P_pred)
        if prev is not None and np.linalg.norm(L - prev) <= 1e-13 * np.linalg.norm(L):
            break
        prev = L.copy()
    G = eye - C @ L.T
    Abar = A @ G
    rho = np.abs(np.linalg.eigvals(Abar)).max()
    # window must annihilate truncated history far below f32 resolution
    assert rho ** WIN < 1e-15, f"decay too slow for WIN={WIN} (rho={rho})"
    SW = np.concatenate([B_W.astype(np.float64) @ G,
                         E_W.astype(np.float64) @ G,
                         L.T], axis=0)                     # [128, NX]
    Apow = np.eye(NX)
    W_cols = []
    for a in range(NA):
        W_cols.append((SW @ Apow).astype(np.float32))
        Apow = Apow @ Abar
    WA = np.concatenate(W_cols, axis=1)                    # [128, NA*128]
    MB = np.zeros((N_CORES, NX, NBW * NX), np.float32)
    for m in range(N_CORES):
        for kl in range(NBW):   # block ascending in t inside the core slice
            e = NA * (NBW - 1 - kl) + TCW * (N_CORES - 1 - m)
            MB[m][:, kl * NX:(kl + 1) * NX] = np.linalg.matrix_power(
                Abar, e).astype(np.float32)
    return WA, MB


def _build_bass():
    import concourse.bacc as bacc
    import concourse.mybir as mybir
    from concourse.tile import TileContext

    f32 = mybir.dt.float32
    nc = bacc.Bacc(None, target_bir_lowering=False)
    # Per-core SBUF-image of the data slice: [128 feat, TCW*B] with column order
    # (q = pos in block ascending t, kl = block, batch).
    zc = nc.dram_tensor("zc", [128, TCW * B], f32, kind="ExternalInput")
    wa = nc.dram_tensor("wa", [128, NA * 128], f32, kind="ExternalInput")
    mb = nc.dram_tensor("mb", [128, NBW * 128], f32, kind="ExternalInput")
    out = nc.dram_tensor("out", [128, B], f32, kind="ExternalOutput")

    NW = NBW * B                        # moving-operand width of inner matmuls
    with TileContext(nc) as tc:
        with (
            tc.tile_pool(name="wpool", bufs=1) as wpool,
            tc.tile_pool(name="zpool", bufs=1) as zpool,
            tc.tile_pool(name="gsb", bufs=1) as gsb_pool,
            tc.tile_pool(name="gpsum", bufs=1, space="PSUM") as gpsum_pool,
            tc.tile_pool(name="ppsum", bufs=1, space="PSUM") as ppsum_pool,
        ):
            # spread the loads across independent DMA rings (sync + scalar
            # HWDGE, gpsimd SWDGE) and order them so the matmul accumulation
            # (emitted q=15 down to 0, i.e. weights W_0,W_1,... first) only
            # ever waits on the half that has already landed.
            w_tile = wpool.tile([128, NA * 128], f32, tag="wa")
            WH = NA * 128 // 2
            zbuf = zpool.tile([128, TCW * B], f32)
            HALF = TCW * B // 2
            nc.scalar.dma_start(out=w_tile[:, :WH], in_=wa[:, :WH])
            nc.gpsimd.dma_start(out=zbuf[:, HALF:], in_=zc[:, HALF:])
            nc.scalar.dma_start(out=w_tile[:, WH:], in_=wa[:, WH:])
            nc.gpsimd.dma_start(out=zbuf[:, :HALF], in_=zc[:, :HALF])
            mb_tile = wpool.tile([128, NBW * 128], f32, tag="mb")
            nc.sync.dma_start(out=mb_tile[:, :], in_=mb[:, :])

            g2 = gpsum_pool.tile([128, NW], f32)
            for q in range(NA - 1, -1, -1):
                # ascending-t position q inside each block uses W_{NA-1-q};
                # PSUM accumulation is order-independent, so run q high->low
                a = NA - 1 - q
                nc.tensor.matmul(
                    g2,
                    w_tile[:, a * 128:(a + 1) * 128],
                    zbuf[:, q * NW:(q + 1) * NW],
                    start=(q == NA - 1), stop=(q == 0),
                )
            g2_sb = gsb_pool.tile([128, NW], f32)
            nc.vector.tensor_copy(out=g2_sb, in_=g2)
            pps = ppsum_pool.tile([128, B], f32)
            for kl in range(NBW):
                nc.tensor.matmul(
                    pps,
                    mb_tile[:, kl * 128:(kl + 1) * 128],
                    g2_sb[:, kl * B:(kl + 1) * B],
                    start=(kl == 0), stop=(kl == NBW - 1),
                )
            tot = gsb_pool.tile([128, B], f32, tag="tot")
            nc.vector.tensor_copy(out=tot, in_=pps)
            nc.sync.dma_start(out=out[:, :], in_=tot[:, :])
    nc.finalize()
    return nc


def _pack_z(Ym, M_flow, DT, D):
    """Per-core SBUF-image arrays [128, TCW*B] (f32, contiguous) for the last
    WIN timesteps.  Column order (q, kl, b); t = (T-WIN) + m*TCW + kl*16 + q."""
    lo = T - WIN
    u = (np.float32(HEAT_C) * M_flow[lo:] * DT[lo:]).astype(np.float32)
    Z = np.concatenate([u, D[lo:], Ym[lo:]], axis=2)   # [WIN, B, 128]
    ZT = Z.transpose(0, 2, 1)                          # [WIN, 128, B] (view)
    Z5 = ZT.reshape(N_CORES, NBW, NA, 128, B)          # (m, kl, q, feat, b)
    Zp = np.ascontiguousarray(Z5.transpose(0, 3, 2, 1, 4))   # (m, feat, q, kl, b)
    return Zp.reshape(N_CORES, 128, TCW * B)


def kernel(Ym, M_flow, DT, D, A_W, B_W, E_W, C_W, Q, R, P0, L0, x0):
    from concourse.bass_utils import run_bass_kernel_spmd

    if "nc" not in _cache:
        _cache["nc"] = _build_bass()
    nc = _cache["nc"]

    WA, MB = _build_weights(A_W, B_W, E_W, C_W, Q, R, P0, L0)
    Zp = _pack_z(Ym, M_flow, DT, D)
    in_maps = [{"zc": Zp[m], "wa": WA, "mb": MB[m]} for m in range(N_CORES)]
    res = run_bass_kernel_spmd(nc, in_maps, core_ids=list(range(N_CORES)))
    xT = np.zeros((NX, B), np.float32)
    for m in range(N_CORES):
        xT += res.results[m]["out"]
    # x0 is zeros in this model; if it were not, its influence decays by
    # Abar^T ~ 0 anyway at f32.
    return np.ascontiguousarray(xT.T)


# revision 13
# speedup vs baseline: 1.0284x; 1.0284x over previous
"""Kalman filter estimator (nn_KalmanFilterEstimator) as a Bass/Tile kernel on 8 TRN2 cores.

Reformulation: the scan is linear in the data once the (data-independent) Riccati
gain sequence is known. With x0 = 0:

    x_{t+1} = x_t @ Aeff_t + c_t,
    c_t     = u_t @ (B_W G_t) + d_t @ (E_W G_t) + ym_t @ Lc_t^T,
    G_t     = I - C_W @ Lc_t^T,   Aeff_t = A_W @ G_t,

so x_T = sum_t c_t @ (Aeff_{t+1} ... Aeff_{T-1}).  The gain converges to Lbar in
~46 steps (rho(Abar) ~ 0.73, checked at runtime), so Aeff_t == Abar beyond the
first few steps and the suffix product is Abar^(T-1-t).  Contributions decay as
rho^age: anything older than ~330 steps underflows to exactly 0 in float32 (the
reference output provably cannot depend on it).  We therefore compute

    x_T = sum_{t >= T-WIN} c_t @ Abar^(T-1-t),        WIN = 128
        (truncation error ~ rho^WIN ~ 5e-18, ~11 orders below the f32
         resolution of the output; checked by assertion at runtime)

time-sharded over 8 cores (16 steps each).  Per core m, with 8-step blocks:

    partial_m = sum_{kl<2} [ sum_{q<8} Z_{t(kl,q)} @ W_{7-q} ] @ MB_{m,kl}
    W_a      = [B_W G; E_W G; Lbar^T] @ Abar^a            ([128 x 128], stacked)
    MB_{m,kl} = Abar^(8 (1-kl) + 16 (7-m))
    Z_t      = [u_t ; d_t ; ym_t] transposed to [128 feat x 128 batch]

All device work is K=128 matmuls accumulated in PSUM (inner stage fuses the two
blocks into N=256 moving operands); the 8 [NX x B] partials are summed on host.
Loads are split across the sync/scalar HWDGE rings and ordered so the reversed
accumulation (q high -> low) only waits on halves that have already landed.
Weight-only precompute (Riccati, matrix powers) runs on host in float64.
"""

import numpy as np

NX, NY, NU, ND = 128, 64, 32, 32
T, B = 2048, 128
HEAT_C = 0.997 * 4185.5 * (1.0 / 3600.0)
N_CORES = 8
WIN = 128                  # time window that fully determines x_T at f32
TCW = WIN // N_CORES       # 16 timesteps per core
NA = 8                     # inner radix (Abar^a, a in [0,8)) = block length
NBW = TCW // NA            # 2 blocks of 8 steps per core
_cache = {}


def _build_weights(A_W, B_W, E_W, C_W, Q, R, P0, L0):
    """Riccati recursion in float64 -> folded steady-state weights (f32)."""
    A = A_W.astype(np.float64); C = C_W.astype(np.float64)
    Qf = Q.astype(np.float64); Rf = R.astype(np.float64)
    eye = np.eye(NX)
    P = P0.astype(np.float64); L = L0.astype(np.float64)
    prev = None
    for t in range(300):
        P_pred = A @ P @ A.T + Qf
        S = Rf + C.T @ P_pred @ C
        L = P_pred @ C @ np.linalg.inv(S)
        P = eye - L @ (C.T @ P_pred)
        if prev is not None and np.linalg.norm(L - prev) <= 1e-13 * np.linalg.norm(L):
            break
        prev = L.copy()
    G = eye - C @ L.T
    Abar = A @ G
    rho = np.abs(np.linalg.eigvals(Abar)).max()
    # window must annihilate truncated history far below f32 resolution
    assert rho ** WIN < 1e-15, f"decay too slow for WIN={WIN} (rho={rho})"
    SW = np.concatenate([B_W.astype(np.float64) @ G,
                         E_W.astype(np.float64) @ G,
                         L.T], axis=0)                     # [128, NX]
    Apow = np.eye(NX)
    W_cols = []
    for a in range(NA):
        W_cols.append((SW @ Apow).astype(np.float32))
        Apow = Apow @ Abar
    WA = np.concatenate(W_cols, axis=1)                    # [128, NA*128]
    MB = np.zeros((N_CORES, NX, NBW * NX), np.float32)
    for m in range(N_CORES):
        for kl in range(NBW):   # block ascending in t inside the core slice
            e = NA * (NBW - 1 - kl) + TCW * (N_CORES - 1 - m)
            MB[m][:, kl * NX:(kl + 1) * NX] = np.linalg.matrix_power(
                Abar, e).astype(np.float32)
    return WA, MB


def _build_bass():
    import concourse.bacc as bacc
    import concourse.mybir as mybir
    from concourse.tile import TileContext

    f32 = mybir.dt.float32
    nc = bacc.Bacc(None, target_bir_lowering=False)
    # Per-core SBUF-image of the data slice: [128 feat, TCW*B] with column order
    # (q = pos in block ascending t, kl = block, batch).
    zc = nc.dram_tensor("zc", [128, TCW * B], f32, kind="ExternalInput")
    wa = nc.dram_tensor("wa", [128, NA * 128], f32, kind="ExternalInput")
    mb = nc.dram_tensor("mb", [128, NBW * 128], f32, kind="ExternalInput")
    out = nc.dram_tensor("out", [128, B], f32, kind="ExternalOutput")

    NW = NBW * B                        # moving-operand width of inner matmuls
    with TileContext(nc) as tc:
        with (
            tc.tile_pool(name="wpool", bufs=1) as wpool,
            tc.tile_pool(name="zpool", bufs=1) as zpool,
            tc.tile_pool(name="gsb", bufs=1) as gsb_pool,
            tc.tile_pool(name="gpsum", bufs=1, space="PSUM") as gpsum_pool,
            tc.tile_pool(name="ppsum", bufs=1, space="PSUM") as ppsum_pool,
        ):
            # spread the loads across independent DMA rings (sync + scalar
            # HWDGE, gpsimd SWDGE) and order them so the matmul accumulation
            # (emitted q=15 down to 0, i.e. weights W_0,W_1,... first) only
            # ever waits on the half that has already landed.
            w_tile = wpool.tile([128, NA * 128], f32, tag="wa")
            WH = NA * 128 // 2
            zbuf = zpool.tile([128, TCW * B], f32)
            HALF = TCW * B // 2
            nc.sync.dma_start(out=w_tile[:, :WH], in_=wa[:, :WH])
            nc.scalar.dma_start(out=zbuf[:, HALF:], in_=zc[:, HALF:])
            nc.sync.dma_start(out=w_tile[:, WH:], in_=wa[:, WH:])
            nc.scalar.dma_start(out=zbuf[:, :HALF], in_=zc[:, :HALF])
            mb_tile = wpool.tile([128, NBW * 128], f32, tag="mb")
            nc.gpsimd.dma_start(out=mb_tile[:, :], in_=mb[:, :])

            g2 = gpsum_pool.tile([128, NW], f32)
            for q in range(NA - 1, -1, -1):
                # ascending-t position q inside each block uses W_{NA-1-q};
                # PSUM accumulation is order-independent, so run q high->low
                a = NA - 1 - q
                nc.tensor.matmul(
                    g2,
                    w_tile[:, a * 128:(a + 1) * 128],
                    zbuf[:, q * NW:(q + 1) * NW],
                    start=(q == NA - 1), stop=(q == 0),
                )
            g2_sb = gsb_pool.tile([128, NW], f32)
            nc.vector.tensor_copy(out=g2_sb, in_=g2)
            pps = ppsum_pool.tile([128, B], f32)
            for kl in range(NBW):
                nc.tensor.matmul(
                    pps,
                    mb_tile[:, kl * 128:(kl + 1) * 128],
                    g2_sb[:, kl * B:(kl + 1) * B],
                    start=(kl == 0), stop=(kl == NBW - 1),
                )
            tot = gsb_pool.tile([128, B], f32, tag="tot")
            nc.vector.tensor_copy(out=tot, in_=pps)
            nc.sync.dma_start(out=out[:, :], in_=tot[:, :])
    nc.finalize()
    return nc


def _pack_z(Ym, M_flow, DT, D):
    """Per-core SBUF-image arrays [128, TCW*B] (f32, contiguous) for the last
    WIN timesteps.  Column order (q, kl, b); t = (T-WIN) + m*TCW + kl*NA + q."""
    lo = T - WIN
    u = (np.float32(HEAT_C) * M_flow[lo:] * DT[lo:]).astype(np.float32)
    Z = np.concatenate([u, D[lo:], Ym[lo:]], axis=2)   # [WIN, B, 128]
    ZT = Z.transpose(0, 2, 1)                          # [WIN, 128, B] (view)
    Z5 = ZT.reshape(N_CORES, NBW, NA, 128, B)          # (m, kl, q, feat, b)
    Zp = np.ascontiguousarray(Z5.transpose(0, 3, 2, 1, 4))   # (m, feat, q, kl, b)
    return Zp.reshape(N_CORES, 128, TCW * B)


def kernel(Ym, M_flow, DT, D, A_W, B_W, E_W, C_W, Q, R, P0, L0, x0):
    from concourse.bass_utils import run_bass_kernel_spmd

    if "nc" not in _cache:
        _cache["nc"] = _build_bass()
    nc = _cache["nc"]

    WA, MB = _build_weights(A_W, B_W, E_W, C_W, Q, R, P0, L0)
    Zp = _pack_z(Ym, M_flow, DT, D)
    in_maps = [{"zc": Zp[m], "wa": WA, "mb": MB[m]} for m in range(N_CORES)]
    res = run_bass_kernel_spmd(nc, in_maps, core_ids=list(range(N_CORES)))
    xT = np.zeros((NX, B), np.float32)
    for m in range(N_CORES):
        xT += res.results[m]["out"]
    # x0 is zeros in this model; if it were not, its influence decays by
    # Abar^T ~ 0 anyway at f32.
    return np.ascontiguousarray(xT.T)


# revision 14
# speedup vs baseline: 1.0818x; 1.0519x over previous
"""Kalman filter estimator (nn_KalmanFilterEstimator) as a Bass/Tile kernel on 8 TRN2 cores.

Reformulation: the scan is linear in the data once the (data-independent) Riccati
gain sequence is known. With x0 = 0:

    x_{t+1} = x_t @ Aeff_t + c_t,
    c_t     = u_t @ (B_W G_t) + d_t @ (E_W G_t) + ym_t @ Lc_t^T,
    G_t     = I - C_W @ Lc_t^T,   Aeff_t = A_W @ G_t,

so x_T = sum_t c_t @ (Aeff_{t+1} ... Aeff_{T-1}).  The gain converges to Lbar in
~46 steps (rho(Abar) ~ 0.73, checked at runtime), so Aeff_t == Abar beyond the
first few steps and the suffix product is Abar^(T-1-t).  Contributions decay as
rho^age: anything older than ~330 steps underflows to exactly 0 in float32 (the
reference output provably cannot depend on it).  We therefore compute

    x_T = sum_{t >= T-WIN} c_t @ Abar^(T-1-t),        WIN = 128
        (truncation error ~ rho^WIN ~ 5e-18, ~11 orders below the f32
         resolution of the output; checked by assertion at runtime)

time-sharded over 8 cores (16 steps each).  Per core m, with 8-step blocks:

    partial_m = sum_{kl<2} [ sum_{q<8} Z_{t(kl,q)} @ W_{7-q} ] @ MB_{m,kl}
    W_a      = [B_W G; E_W G; Lbar^T] @ Abar^a            ([128 x 128], stacked)
    MB_{m,kl} = Abar^(8 (1-kl) + 16 (7-m))
    Z_t      = [u_t ; d_t ; ym_t] transposed to [128 feat x 128 batch]

All device work is K=128 matmuls accumulated in PSUM (inner stage fuses the two
blocks into N=256 moving operands); the 8 [NX x B] partials are summed on host.
Loads are split across the sync/scalar HWDGE rings and ordered so the reversed
accumulation (q high -> low) only waits on halves that have already landed.
Weight-only precompute (Riccati, matrix powers) runs on host in float64.
"""

import numpy as np

NX, NY, NU, ND = 128, 64, 32, 32
T, B = 2048, 128
HEAT_C = 0.997 * 4185.5 * (1.0 / 3600.0)
N_CORES = 8
WIN = 128                  # time window that fully determines x_T at f32
TCW = WIN // N_CORES       # 16 timesteps per core
NA = 8                     # inner radix (Abar^a, a in [0,8)) = block length
NBW = TCW // NA            # 2 blocks of 8 steps per core
_cache = {}


def _build_weights(A_W, B_W, E_W, C_W, Q, R, P0, L0):
    """Riccati recursion in float64 -> folded steady-state weights (f32)."""
    A = A_W.astype(np.float64); C = C_W.astype(np.float64)
    Qf = Q.astype(np.float64); Rf = R.astype(np.float64)
    eye = np.eye(NX)
    P = P0.astype(np.float64); L = L0.astype(np.float64)
    prev = None
    for t in range(300):
        P_pred = A @ P @ A.T + Qf
        S = Rf + C.T @ P_pred @ C
        L = P_pred @ C @ np.linalg.inv(S)
        P = eye - L @ (C.T @ P_pred)
        if prev is not None and np.linalg.norm(L - prev) <= 1e-13 * np.linalg.norm(L):
            break
        prev = L.copy()
    G = eye - C @ L.T
    Abar = A @ G
    rho = np.abs(np.linalg.eigvals(Abar)).max()
    # window must annihilate truncated history far below f32 resolution
    assert rho ** WIN < 1e-15, f"decay too slow for WIN={WIN} (rho={rho})"
    SW = np.concatenate([B_W.astype(np.float64) @ G,
                         E_W.astype(np.float64) @ G,
                         L.T], axis=0)                     # [128, NX]
    Apow = np.eye(NX)
    W_cols = []
    for a in range(NA):
        W_cols.append((SW @ Apow).astype(np.float32))
        Apow = Apow @ Abar
    WA = np.concatenate(W_cols, axis=1)                    # [128, NA*128]
    MB = np.zeros((N_CORES, NX, NBW * NX), np.float32)
    for m in range(N_CORES):
        for kl in range(NBW):   # block ascending in t inside the core slice
            e = NA * (NBW - 1 - kl) + TCW * (N_CORES - 1 - m)
            MB[m][:, kl * NX:(kl + 1) * NX] = np.linalg.matrix_power(
                Abar, e).astype(np.float32)
    return WA, MB


def _build_bass():
    import concourse.bacc as bacc
    import concourse.mybir as mybir
    from concourse.tile import TileContext

    f32 = mybir.dt.float32
    nc = bacc.Bacc(None, target_bir_lowering=False)
    # Per-core SBUF-image of the data slice: [128 feat, TCW*B] with column order
    # (q = pos in block ascending t, kl = block, batch).
    zc = nc.dram_tensor("zc", [128, TCW * B], f32, kind="ExternalInput")
    wa = nc.dram_tensor("wa", [128, NA * 128], f32, kind="ExternalInput")
    mb = nc.dram_tensor("mb", [128, NBW * 128], f32, kind="ExternalInput")
    out = nc.dram_tensor("out", [128, B], f32, kind="ExternalOutput")

    NW = NBW * B                        # moving-operand width of inner matmuls
    with TileContext(nc) as tc:
        with (
            tc.tile_pool(name="wpool", bufs=1) as wpool,
            tc.tile_pool(name="zpool", bufs=1) as zpool,
            tc.tile_pool(name="gsb", bufs=1) as gsb_pool,
            tc.tile_pool(name="gpsum", bufs=1, space="PSUM") as gpsum_pool,
            tc.tile_pool(name="ppsum", bufs=1, space="PSUM") as ppsum_pool,
        ):
            # spread the loads across independent DMA rings (sync + scalar
            # HWDGE, gpsimd SWDGE) and order them so the matmul accumulation
            # (emitted q=7 down to 0, i.e. weights W_0,W_1,... first) only
            # ever waits on the half that has already landed.
            w_tile = wpool.tile([128, NA * 128], f32, tag="wa")
            WH = NA * 128 // 2
            zbuf = zpool.tile([128, TCW * B], f32)
            HALF = TCW * B // 2
            nc.sync.dma_start(out=w_tile[:, :WH], in_=wa[:, :WH])
            nc.scalar.dma_start(out=zbuf[:, HALF:], in_=zc[:, HALF:])
            nc.sync.dma_start(out=w_tile[:, WH:], in_=wa[:, WH:])
            nc.scalar.dma_start(out=zbuf[:, :HALF], in_=zc[:, :HALF])
            mb_tile = wpool.tile([128, NBW * 128], f32, tag="mb")
            nc.gpsimd.dma_start(out=mb_tile[:, :], in_=mb[:, :])

            g2 = gpsum_pool.tile([128, NW], f32)
            for q in range(NA - 1, -1, -1):
                # ascending-t position q inside each block uses W_{NA-1-q};
                # PSUM accumulation is order-independent, so run q high->low
                a = NA - 1 - q
                nc.tensor.matmul(
                    g2,
                    w_tile[:, a * 128:(a + 1) * 128],
                    zbuf[:, q * NW:(q + 1) * NW],
                    start=(q == NA - 1), stop=(q == 0),
                )
            g2_sb = gsb_pool.tile([128, NW], f32)
            nc.vector.tensor_copy(out=g2_sb, in_=g2)
            pps = ppsum_pool.tile([128, B], f32)
            for kl in range(NBW):
                nc.tensor.matmul(
                    pps,
                    mb_tile[:, kl * 128:(kl + 1) * 128],
                    g2_sb[:, kl * B:(kl + 1) * B],
                    start=(kl == 0), stop=(kl == NBW - 1),
                )
            tot = gsb_pool.tile([128, B], f32, tag="tot")
            nc.vector.tensor_copy(out=tot, in_=pps)
            nc.sync.dma_start(out=out[:, :], in_=tot[:, :])
    nc.finalize()
    return nc


def _pack_z(Ym, M_flow, DT, D):
    """Per-core SBUF-image arrays [128, TCW*B] (f32, contiguous) for the last
    WIN timesteps.  Column order (q, kl, b); t = (T-WIN) + m*TCW + kl*NA + q."""
    lo = T - WIN
    u = (np.float32(HEAT_C) * M_flow[lo:] * DT[lo:]).astype(np.float32)
    Z = np.concatenate([u, D[lo:], Ym[lo:]], axis=2)   # [WIN, B, 128]
    ZT = Z.transpose(0, 2, 1)                          # [WIN, 128, B] (view)
    Z5 = ZT.reshape(N_CORES, NBW, NA, 128, B)          # (m, kl, q, feat, b)
    Zp = np.ascontiguousarray(Z5.transpose(0, 3, 2, 1, 4))   # (m, feat, q, kl, b)
    return Zp.reshape(N_CORES, 128, TCW * B)


def kernel(Ym, M_flow, DT, D, A_W, B_W, E_W, C_W, Q, R, P0, L0, x0):
    from concourse.bass_utils import run_bass_kernel_spmd

    if "nc" not in _cache:
        _cache["nc"] = _build_bass()
    nc = _cache["nc"]

    WA, MB = _build_weights(A_W, B_W, E_W, C_W, Q, R, P0, L0)
    Zp = _pack_z(Ym, M_flow, DT, D)
    in_maps = [{"zc": Zp[m], "wa": WA, "mb": MB[m]} for m in range(N_CORES)]
    res = run_bass_kernel_spmd(nc, in_maps, core_ids=list(range(N_CORES)))
    xT = np.zeros((NX, B), np.float32)
    for m in range(N_CORES):
        xT += res.results[m]["out"]
    # x0 is zeros in this model; if it were not, its influence decays by
    # Abar^T ~ 0 anyway at f32.
    return np.ascontiguousarray(xT.T)


# revision 15
# speedup vs baseline: 1.1828x; 1.0934x over previous
"""Kalman filter estimator (nn_KalmanFilterEstimator) as a Bass/Tile kernel on 8 TRN2 cores.

Reformulation: the scan is linear in the data once the (data-independent) Riccati
gain sequence is known. With x0 = 0:

    x_{t+1} = x_t @ Aeff_t + c_t,
    c_t     = u_t @ (B_W G_t) + d_t @ (E_W G_t) + ym_t @ Lc_t^T,
    G_t     = I - C_W @ Lc_t^T,   Aeff_t = A_W @ G_t,

so x_T = sum_t c_t @ (Aeff_{t+1} ... Aeff_{T-1}).  The gain converges to Lbar in
~46 steps (rho(Abar) ~ 0.73, checked at runtime), so Aeff_t == Abar beyond the
first few steps and the suffix product is Abar^(T-1-t).  Contributions decay as
rho^age: anything older than ~330 steps underflows to exactly 0 in float32 (the
reference output provably cannot depend on it).  We therefore compute

    x_T = sum_{t >= T-WIN} c_t @ Abar^(T-1-t),        WIN = 64
        (exact dropped-tail measured at 1.2e-9 relative -- 400x below the
         ~5e-7 f32 arithmetic noise; decay checked by assertion at runtime)

time-sharded over 8 cores (8 steps each).  Per core m, with 8-step blocks:

    partial_m = sum_{kl<2} [ sum_{q<8} Z_{t(kl,q)} @ W_{7-q} ] @ MB_{m,kl}
    W_a      = [B_W G; E_W G; Lbar^T] @ Abar^a            ([128 x 128], stacked)
    MB_{m,kl} = Abar^(8 (1-kl) + 16 (7-m))
    Z_t      = [u_t ; d_t ; ym_t] transposed to [128 feat x 128 batch]

All device work is K=128 matmuls accumulated in PSUM (inner stage fuses the two
blocks into N=256 moving operands); the 8 [NX x B] partials are summed on host.
Loads are split across the sync/scalar HWDGE rings and ordered so the reversed
accumulation (q high -> low) only waits on halves that have already landed.
Weight-only precompute (Riccati, matrix powers) runs on host in float64.
"""

import numpy as np

NX, NY, NU, ND = 128, 64, 32, 32
T, B = 2048, 128
HEAT_C = 0.997 * 4185.5 * (1.0 / 3600.0)
N_CORES = 8
WIN = 64                   # time window that fully determines x_T at f32
TCW = WIN // N_CORES       # 8 timesteps per core
NA = 8                     # inner radix (Abar^a, a in [0,8)) = block length
NBW = TCW // NA            # 1 block of 8 steps per core
_cache = {}


def _build_weights(A_W, B_W, E_W, C_W, Q, R, P0, L0):
    """Riccati recursion in float64 -> folded steady-state weights (f32)."""
    A = A_W.astype(np.float64); C = C_W.astype(np.float64)
    Qf = Q.astype(np.float64); Rf = R.astype(np.float64)
    eye = np.eye(NX)
    P = P0.astype(np.float64); L = L0.astype(np.float64)
    prev = None
    for t in range(300):
        P_pred = A @ P @ A.T + Qf
        S = Rf + C.T @ P_pred @ C
        L = P_pred @ C @ np.linalg.inv(S)
        P = eye - L @ (C.T @ P_pred)
        if prev is not None and np.linalg.norm(L - prev) <= 1e-13 * np.linalg.norm(L):
            break
        prev = L.copy()
    G = eye - C @ L.T
    Abar = A @ G
    rho = np.abs(np.linalg.eigvals(Abar)).max()
    # window must annihilate truncated history below f32 resolution of the
    # output (measured dropped-tail 1.2e-9 rel vs 5e-7 f32 arithmetic noise)
    assert rho ** WIN < 1e-8, f"decay too slow for WIN={WIN} (rho={rho})"
    SW = np.concatenate([B_W.astype(np.float64) @ G,
                         E_W.astype(np.float64) @ G,
                         L.T], axis=0)                     # [128, NX]
    Apow = np.eye(NX)
    W_cols = []
    for a in range(NA):
        W_cols.append((SW @ Apow).astype(np.float32))
        Apow = Apow @ Abar
    WA = np.concatenate(W_cols, axis=1)                    # [128, NA*128]
    MB = np.zeros((N_CORES, NX, NBW * NX), np.float32)
    for m in range(N_CORES):
        for kl in range(NBW):   # block ascending in t inside the core slice
            e = NA * (NBW - 1 - kl) + TCW * (N_CORES - 1 - m)
            MB[m][:, kl * NX:(kl + 1) * NX] = np.linalg.matrix_power(
                Abar, e).astype(np.float32)
    return WA, MB


def _build_bass():
    import concourse.bacc as bacc
    import concourse.mybir as mybir
    from concourse.tile import TileContext

    f32 = mybir.dt.float32
    nc = bacc.Bacc(None, target_bir_lowering=False)
    # Per-core SBUF-image of the data slice: [128 feat, TCW*B] with column order
    # (q = pos in block ascending t, kl = block, batch).
    zc = nc.dram_tensor("zc", [128, TCW * B], f32, kind="ExternalInput")
    wa = nc.dram_tensor("wa", [128, NA * 128], f32, kind="ExternalInput")
    mb = nc.dram_tensor("mb", [128, NBW * 128], f32, kind="ExternalInput")
    out = nc.dram_tensor("out", [128, B], f32, kind="ExternalOutput")

    NW = NBW * B                        # moving-operand width of inner matmuls
    with TileContext(nc) as tc:
        with (
            tc.tile_pool(name="wpool", bufs=1) as wpool,
            tc.tile_pool(name="zpool", bufs=1) as zpool,
            tc.tile_pool(name="gsb", bufs=1) as gsb_pool,
            tc.tile_pool(name="gpsum", bufs=1, space="PSUM") as gpsum_pool,
            tc.tile_pool(name="ppsum", bufs=1, space="PSUM") as ppsum_pool,
        ):
            # spread the loads across independent DMA rings (sync + scalar
            # HWDGE, gpsimd SWDGE) and order them so the matmul accumulation
            # (emitted q=7 down to 0, i.e. weights W_0,W_1,... first) only
            # ever waits on the half that has already landed.
            w_tile = wpool.tile([128, NA * 128], f32, tag="wa")
            WH = NA * 128 // 2
            zbuf = zpool.tile([128, TCW * B], f32)
            HALF = TCW * B // 2
            nc.scalar.dma_start(out=w_tile[:, :WH], in_=wa[:, :WH])
            nc.sync.dma_start(out=w_tile[:, WH:], in_=wa[:, WH:])
            nc.scalar.dma_start(out=zbuf[:, HALF:], in_=zc[:, HALF:])
            nc.scalar.dma_start(out=zbuf[:, :HALF], in_=zc[:, :HALF])
            mb_tile = wpool.tile([128, NBW * 128], f32, tag="mb")
            nc.sync.dma_start(out=mb_tile[:, :], in_=mb[:, :])

            g2 = gpsum_pool.tile([128, NW], f32)
            for q in range(NA - 1, -1, -1):
                # ascending-t position q inside each block uses W_{NA-1-q};
                # PSUM accumulation is order-independent, so run q high->low
                a = NA - 1 - q
                nc.tensor.matmul(
                    g2,
                    w_tile[:, a * 128:(a + 1) * 128],
                    zbuf[:, q * NW:(q + 1) * NW],
                    start=(q == NA - 1), stop=(q == 0),
                )
            g2_sb = gsb_pool.tile([128, NW], f32)
            nc.vector.tensor_copy(out=g2_sb, in_=g2)
            pps = ppsum_pool.tile([128, B], f32)
            for kl in range(NBW):
                nc.tensor.matmul(
                    pps,
                    mb_tile[:, kl * 128:(kl + 1) * 128],
                    g2_sb[:, kl * B:(kl + 1) * B],
                    start=(kl == 0), stop=(kl == NBW - 1),
                )
            tot = gsb_pool.tile([128, B], f32, tag="tot")
            nc.vector.tensor_copy(out=tot, in_=pps)
            nc.sync.dma_start(out=out[:, :], in_=tot[:, :])
    nc.finalize()
    return nc


def _pack_z(Ym, M_flow, DT, D):
    """Per-core SBUF-image arrays [128, TCW*B] (f32, contiguous) for the last
    WIN timesteps.  Column order (q, kl, b); t = (T-WIN) + m*TCW + kl*NA + q."""
    lo = T - WIN
    u = (np.float32(HEAT_C) * M_flow[lo:] * DT[lo:]).astype(np.float32)
    Z = np.concatenate([u, D[lo:], Ym[lo:]], axis=2)   # [WIN, B, 128]
    ZT = Z.transpose(0, 2, 1)                          # [WIN, 128, B] (view)
    Z5 = ZT.reshape(N_CORES, NBW, NA, 128, B)          # (m, kl, q, feat, b)
    Zp = np.ascontiguousarray(Z5.transpose(0, 3, 2, 1, 4))   # (m, feat, q, kl, b)
    return Zp.reshape(N_CORES, 128, TCW * B)


def kernel(Ym, M_flow, DT, D, A_W, B_W, E_W, C_W, Q, R, P0, L0, x0):
    from concourse.bass_utils import run_bass_kernel_spmd

    if "nc" not in _cache:
        _cache["nc"] = _build_bass()
    nc = _cache["nc"]

    WA, MB = _build_weights(A_W, B_W, E_W, C_W, Q, R, P0, L0)
    Zp = _pack_z(Ym, M_flow, DT, D)
    in_maps = [{"zc": Zp[m], "wa": WA, "mb": MB[m]} for m in range(N_CORES)]
    res = run_bass_kernel_spmd(nc, in_maps, core_ids=list(range(N_CORES)))
    xT = np.zeros((NX, B), np.float32)
    for m in range(N_CORES):
        xT += res.results[m]["out"]
    # x0 is zeros in this model; if it were not, its influence decays by
    # Abar^T ~ 0 anyway at f32.
    return np.ascontiguousarray(xT.T)


# revision 16
# speedup vs baseline: 1.2642x; 1.0688x over previous
"""Kalman filter estimator (nn_KalmanFilterEstimator) as a Bass/Tile kernel on 8 TRN2 cores.

Reformulation: the scan is linear in the data once the (data-independent) Riccati
gain sequence is known. With x0 = 0:

    x_{t+1} = x_t @ Aeff_t + c_t,
    c_t     = u_t @ (B_W G_t) + d_t @ (E_W G_t) + ym_t @ Lc_t^T,
    G_t     = I - C_W @ Lc_t^T,   Aeff_t = A_W @ G_t,

so x_T = sum_t c_t @ (Aeff_{t+1} ... Aeff_{T-1}).  The gain converges to Lbar in
~46 steps (rho(Abar) ~ 0.73, checked at runtime), so Aeff_t == Abar beyond the
first few steps and the suffix product is Abar^(T-1-t).  Contributions decay as
rho^age: anything older than ~330 steps underflows to exactly 0 in float32 (the
reference output provably cannot depend on it).  We therefore compute

    x_T = sum_{t >= T-WIN} c_t @ Abar^(T-1-t),        WIN = 64
        (exact dropped-tail measured at 1.2e-9 relative -- 400x below the
         ~5e-7 f32 arithmetic noise; decay checked by assertion at runtime)

time-sharded over 8 cores (8 steps each).  Per core m, with 8-step blocks:

    partial_m = sum_{q<8} Z_{t(m,q)} @ W'_{m,7-q}
    W'_{m,a} = [B_W G; E_W G; Lbar^T] @ Abar^(a + 8 (7-m))   ([128 x 128])
    Z_t      = [u_t ; d_t ; ym_t] transposed to [128 feat x 128 batch]

All device work is 8 K=128 matmuls accumulated in one PSUM tile per core
(the per-core outer power is folded into the weights on host, so there is no
combine stage); the 8 [NX x B] partials are summed on host.
Loads are split across the sync/scalar HWDGE rings and ordered so the reversed
accumulation (q high -> low) only waits on halves that have already landed.
Weight-only precompute (Riccati, matrix powers) runs on host in float64.
"""

import numpy as np

NX, NY, NU, ND = 128, 64, 32, 32
T, B = 2048, 128
HEAT_C = 0.997 * 4185.5 * (1.0 / 3600.0)
N_CORES = 8
WIN = 64                   # time window that fully determines x_T at f32
TCW = WIN // N_CORES       # 8 timesteps per core
NA = 8                     # inner radix (Abar^a, a in [0,8)) = block length
NBW = TCW // NA            # 1 block of 8 steps per core
_cache = {}


def _build_weights(A_W, B_W, E_W, C_W, Q, R, P0, L0):
    """Riccati recursion in float64 -> folded steady-state weights (f32)."""
    A = A_W.astype(np.float64); C = C_W.astype(np.float64)
    Qf = Q.astype(np.float64); Rf = R.astype(np.float64)
    eye = np.eye(NX)
    P = P0.astype(np.float64); L = L0.astype(np.float64)
    prev = None
    for t in range(300):
        P_pred = A @ P @ A.T + Qf
        S = Rf + C.T @ P_pred @ C
        L = P_pred @ C @ np.linalg.inv(S)
        P = eye - L @ (C.T @ P_pred)
        if prev is not None and np.linalg.norm(L - prev) <= 1e-13 * np.linalg.norm(L):
            break
        prev = L.copy()
    G = eye - C @ L.T
    Abar = A @ G
    rho = np.abs(np.linalg.eigvals(Abar)).max()
    # window must annihilate truncated history below f32 resolution of the
    # output (measured dropped-tail 1.2e-9 rel vs 5e-7 f32 arithmetic noise)
    assert rho ** WIN < 1e-8, f"decay too slow for WIN={WIN} (rho={rho})"
    SW = np.concatenate([B_W.astype(np.float64) @ G,
                         E_W.astype(np.float64) @ G,
                         L.T], axis=0)                     # [128, NX]
    # fold the per-core outer power Abar^(TCW*(7-m)) straight into the
    # stacked weights: per core only 8 [128,128] lhsT matrices, no combine
    WA = np.zeros((N_CORES, NX, NA * NX), np.float32)
    for m in range(N_CORES):
        outer = np.linalg.matrix_power(Abar, TCW * (N_CORES - 1 - m))
        Apow = np.eye(NX)
        for a in range(NA):
            WA[m][:, a * NX:(a + 1) * NX] = (SW @ Apow @ outer).astype(np.float32)
            Apow = Apow @ Abar
    return WA


def _build_bass():
    import concourse.bacc as bacc
    import concourse.mybir as mybir
    from concourse.tile import TileContext

    f32 = mybir.dt.float32
    nc = bacc.Bacc(None, target_bir_lowering=False)
    # Per-core SBUF-image of the data slice: [128 feat, TCW*B] with column order
    # (q = pos in block ascending t, kl = block, batch).
    zc = nc.dram_tensor("zc", [128, TCW * B], f32, kind="ExternalInput")
    wa = nc.dram_tensor("wa", [128, NA * 128], f32, kind="ExternalInput")
    out = nc.dram_tensor("out", [128, B], f32, kind="ExternalOutput")

    NW = NBW * B                        # moving-operand width of inner matmuls
    with TileContext(nc) as tc:
        with (
            tc.tile_pool(name="wpool", bufs=1) as wpool,
            tc.tile_pool(name="zpool", bufs=1) as zpool,
            tc.tile_pool(name="gsb", bufs=1) as gsb_pool,
            tc.tile_pool(name="gpsum", bufs=1, space="PSUM") as gpsum_pool,
            tc.tile_pool(name="ppsum", bufs=1, space="PSUM") as ppsum_pool,
        ):
            # spread the loads across independent DMA rings (sync + scalar
            # HWDGE, gpsimd SWDGE) and order them so the matmul accumulation
            # (emitted q=7 down to 0, i.e. weights W_0,W_1,... first) only
            # ever waits on the half that has already landed.
            w_tile = wpool.tile([128, NA * 128], f32, tag="wa")
            WH = NA * 128 // 2
            zbuf = zpool.tile([128, TCW * B], f32)
            HALF = TCW * B // 2
            nc.scalar.dma_start(out=w_tile[:, :WH], in_=wa[:, :WH])
            nc.sync.dma_start(out=w_tile[:, WH:], in_=wa[:, WH:])
            nc.scalar.dma_start(out=zbuf[:, HALF:], in_=zc[:, HALF:])
            nc.scalar.dma_start(out=zbuf[:, :HALF], in_=zc[:, :HALF])

            pps = ppsum_pool.tile([128, B], f32)
            for q in range(NA - 1, -1, -1):
                # ascending-t position q uses W'_{NA-1-q} (outer power folded
                # in on host); PSUM accumulation is order-independent, so run
                # q high->low to chase the DMA halves
                a = NA - 1 - q
                nc.tensor.matmul(
                    pps,
                    w_tile[:, a * 128:(a + 1) * 128],
                    zbuf[:, q * NW:(q + 1) * NW],
                    start=(q == NA - 1), stop=(q == 0),
                )
            tot = gsb_pool.tile([128, B], f32, tag="tot")
            nc.vector.tensor_copy(out=tot, in_=pps)
            nc.sync.dma_start(out=out[:, :], in_=tot[:, :])
    nc.finalize()
    return nc


def _pack_z(Ym, M_flow, DT, D):
    """Per-core SBUF-image arrays [128, TCW*B] (f32, contiguous) for the last
    WIN timesteps.  Column order (q, kl, b); t = (T-WIN) + m*TCW + kl*NA + q."""
    lo = T - WIN
    u = (np.float32(HEAT_C) * M_flow[lo:] * DT[lo:]).astype(np.float32)
    Z = np.concatenate([u, D[lo:], Ym[lo:]], axis=2)   # [WIN, B, 128]
    ZT = Z.transpose(0, 2, 1)                          # [WIN, 128, B] (view)
    Z5 = ZT.reshape(N_CORES, NBW, NA, 128, B)          # (m, kl, q, feat, b)
    Zp = np.ascontiguousarray(Z5.transpose(0, 3, 2, 1, 4))   # (m, feat, q, kl, b)
    return Zp.reshape(N_CORES, 128, TCW * B)


def kernel(Ym, M_flow, DT, D, A_W, B_W, E_W, C_W, Q, R, P0, L0, x0):
    from concourse.bass_utils import run_bass_kernel_spmd

    if "nc" not in _cache:
        _cache["nc"] = _build_bass()
    nc = _cache["nc"]

    WA = _build_weights(A_W, B_W, E_W, C_W, Q, R, P0, L0)
    Zp = _pack_z(Ym, M_flow, DT, D)
    in_maps = [{"zc": Zp[m], "wa": WA[m]} for m in range(N_CORES)]
    res = run_bass_kernel_spmd(nc, in_maps, core_ids=list(range(N_CORES)))
    xT = np.zeros((NX, B), np.float32)
    for m in range(N_CORES):
        xT += res.results[m]["out"]
    # x0 is zeros in this model; if it were not, its influence decays by
    # Abar^T ~ 0 anyway at f32.
    return np.ascontiguousarray(xT.T)


# revision 17
# speedup vs baseline: 1.3147x; 1.0400x over previous
"""Kalman filter estimator (nn_KalmanFilterEstimator) as a Bass/Tile kernel on 8 TRN2 cores.

Reformulation: the scan is linear in the data once the (data-independent) Riccati
gain sequence is known. With x0 = 0:

    x_{t+1} = x_t @ Aeff_t + c_t,
    c_t     = u_t @ (B_W G_t) + d_t @ (E_W G_t) + ym_t @ Lc_t^T,
    G_t     = I - C_W @ Lc_t^T,   Aeff_t = A_W @ G_t,

so x_T = sum_t c_t @ (Aeff_{t+1} ... Aeff_{T-1}).  The gain converges to Lbar in
~46 steps (rho(Abar) ~ 0.73, checked at runtime), so Aeff_t == Abar beyond the
first few steps and the suffix product is Abar^(T-1-t).  Contributions decay as
rho^age: anything older than ~330 steps underflows to exactly 0 in float32 (the
reference output provably cannot depend on it).  We therefore compute

    x_T = sum_{t >= T-WIN} c_t @ Abar^(T-1-t),        WIN = 64
        (exact dropped-tail measured at 1.2e-9 relative -- 400x below the
         ~5e-7 f32 arithmetic noise; decay checked by assertion at runtime)

time-sharded over 8 cores (8 steps each).  Per core m, with 8-step blocks:

    partial_m = sum_{q<8} Z_{t(m,q)} @ W'_{m,7-q}
    W'_{m,a} = [B_W G; E_W G; Lbar^T] @ Abar^(a + 8 (7-m))   ([128 x 128])
    Z_t      = [u_t ; d_t ; ym_t] transposed to [128 feat x 128 batch]

All device work is 8 K=128 matmuls accumulated in one PSUM tile per core
(the per-core outer power is folded into the weights on host, so there is no
combine stage); the 8 [NX x B] partials are summed on host.
Loads are split across the sync/scalar HWDGE rings and ordered so the reversed
accumulation (q high -> low) only waits on halves that have already landed.
Weight-only precompute (Riccati, matrix powers) runs on host in float64.
"""

import numpy as np

NX, NY, NU, ND = 128, 64, 32, 32
T, B = 2048, 128
HEAT_C = 0.997 * 4185.5 * (1.0 / 3600.0)
N_CORES = 8
WIN = 64                   # time window that fully determines x_T at f32
TCW = WIN // N_CORES       # 8 timesteps per core
NA = 8                     # inner radix (Abar^a, a in [0,8)) = block length
NBW = TCW // NA            # 1 block of 8 steps per core
_cache = {}


def _build_weights(A_W, B_W, E_W, C_W, Q, R, P0, L0):
    """Riccati recursion in float64 -> folded steady-state weights (f32)."""
    A = A_W.astype(np.float64); C = C_W.astype(np.float64)
    Qf = Q.astype(np.float64); Rf = R.astype(np.float64)
    eye = np.eye(NX)
    P = P0.astype(np.float64); L = L0.astype(np.float64)
    prev = None
    for t in range(300):
        P_pred = A @ P @ A.T + Qf
        S = Rf + C.T @ P_pred @ C
        L = P_pred @ C @ np.linalg.inv(S)
        P = eye - L @ (C.T @ P_pred)
        if prev is not None and np.linalg.norm(L - prev) <= 1e-13 * np.linalg.norm(L):
            break
        prev = L.copy()
    G = eye - C @ L.T
    Abar = A @ G
    rho = np.abs(np.linalg.eigvals(Abar)).max()
    # window must annihilate truncated history below f32 resolution of the
    # output (measured dropped-tail 1.2e-9 rel vs 5e-7 f32 arithmetic noise)
    assert rho ** WIN < 1e-8, f"decay too slow for WIN={WIN} (rho={rho})"
    SW = np.concatenate([B_W.astype(np.float64) @ G,
                         E_W.astype(np.float64) @ G,
                         L.T], axis=0)                     # [128, NX]
    # fold the per-core outer power Abar^(TCW*(7-m)) straight into the
    # stacked weights: per core only 8 [128,128] lhsT matrices, no combine
    WA = np.zeros((N_CORES, NX, NA * NX), np.float32)
    for m in range(N_CORES):
        outer = np.linalg.matrix_power(Abar, TCW * (N_CORES - 1 - m))
        Apow = np.eye(NX)
        for a in range(NA):
            WA[m][:, a * NX:(a + 1) * NX] = (SW @ Apow @ outer).astype(np.float32)
            Apow = Apow @ Abar
    return WA


def _build_bass():
    import concourse.bacc as bacc
    import concourse.mybir as mybir
    from concourse.tile import TileContext

    f32 = mybir.dt.float32
    nc = bacc.Bacc(None, target_bir_lowering=False)
    # Per-core SBUF-image of the data slice: [128 feat, TCW*B] with column order
    # (q = pos in block ascending t, kl = block, batch).
    # weights and data interleaved in execution order: 16 chunks of 128 cols
    # [W'_0 | z_{q=7} | W'_1 | z_{q=6} | ... ] so the two half-loads land in
    # exactly the order the PSUM accumulation consumes them
    wz = nc.dram_tensor("wz", [128, 2 * NA * 128], f32, kind="ExternalInput")
    out = nc.dram_tensor("out", [128, B], f32, kind="ExternalOutput")

    NW = NBW * B                        # moving-operand width of inner matmuls
    with TileContext(nc) as tc:
        with (
            tc.tile_pool(name="wpool", bufs=1) as wpool,
            tc.tile_pool(name="zpool", bufs=1) as zpool,
            tc.tile_pool(name="gsb", bufs=1) as gsb_pool,
            tc.tile_pool(name="gpsum", bufs=1, space="PSUM") as gpsum_pool,
            tc.tile_pool(name="ppsum", bufs=1, space="PSUM") as ppsum_pool,
        ):
            # two half-loads on the fast scalar HWDGE ring; the accumulation
            # (emitted i=0..7 = W'_i with z_{q=7-i}) consumes chunks in ring
            # order, so the first four matmuls only wait on the first half
            wz_tile = zpool.tile([128, 2 * NA * 128], f32, tag="wz")
            WZH = NA * 128
            nc.scalar.dma_start(out=wz_tile[:, :WZH], in_=wz[:, :WZH])
            nc.scalar.dma_start(out=wz_tile[:, WZH:], in_=wz[:, WZH:])

            pps = ppsum_pool.tile([128, B], f32)
            for i in range(NA):
                # chunk 2i = lhsT W'_i, chunk 2i+1 = moving z_{q=NA-1-i};
                # PSUM accumulation is order-independent
                nc.tensor.matmul(
                    pps,
                    wz_tile[:, (2 * i) * 128:(2 * i + 1) * 128],
                    wz_tile[:, (2 * i + 1) * 128:(2 * i + 2) * 128],
                    start=(i == 0), stop=(i == NA - 1),
                )
            tot = gsb_pool.tile([128, B], f32, tag="tot")
            nc.vector.tensor_copy(out=tot, in_=pps)
            nc.scalar.dma_start(out=out[:, :], in_=tot[:, :])
    nc.finalize()
    return nc


def _pack_z(Ym, M_flow, DT, D):
    """Per-core SBUF-image arrays [128, TCW*B] (f32, contiguous) for the last
    WIN timesteps.  Column order (q, kl, b); t = (T-WIN) + m*TCW + kl*NA + q."""
    lo = T - WIN
    u = (np.float32(HEAT_C) * M_flow[lo:] * DT[lo:]).astype(np.float32)
    Z = np.concatenate([u, D[lo:], Ym[lo:]], axis=2)   # [WIN, B, 128]
    ZT = Z.transpose(0, 2, 1)                          # [WIN, 128, B] (view)
    Z5 = ZT.reshape(N_CORES, NBW, NA, 128, B)          # (m, kl, q, feat, b)
    Zp = np.ascontiguousarray(Z5.transpose(0, 3, 2, 1, 4))   # (m, feat, q, kl, b)
    return Zp.reshape(N_CORES, 128, TCW * B)


def kernel(Ym, M_flow, DT, D, A_W, B_W, E_W, C_W, Q, R, P0, L0, x0):
    from concourse.bass_utils import run_bass_kernel_spmd

    if "nc" not in _cache:
        _cache["nc"] = _build_bass()
    nc = _cache["nc"]

    WA = _build_weights(A_W, B_W, E_W, C_W, Q, R, P0, L0)
    Zp = _pack_z(Ym, M_flow, DT, D)
    WZ = np.zeros((N_CORES, 128, 2 * NA * 128), np.float32)
    for i in range(NA):
        q = NA - 1 - i
        WZ[:, :, (2 * i) * 128:(2 * i + 1) * 128] = WA[:, :, i * 128:(i + 1) * 128]
        WZ[:, :, (2 * i + 1) * 128:(2 * i + 2) * 128] = Zp[:, :, q * B:(q + 1) * B]
    in_maps = [{"wz": WZ[m]} for m in range(N_CORES)]
    res = run_bass_kernel_spmd(nc, in_maps, core_ids=list(range(N_CORES)))
    xT = np.zeros((NX, B), np.float32)
    for m in range(N_CORES):
        xT += res.results[m]["out"]
    # x0 is zeros in this model; if it were not, its influence decays by
    # Abar^T ~ 0 anyway at f32.
    return np.ascontiguousarray(xT.T)
